# revision 20
# baseline (speedup 1.0000x reference)
"""Trainium2 Bass kernel for nn_Encoder_506806141403.

12-layer transformer encoder (D=768, H=12, FF=3072) with the quirk that
attention scores use Q vs V (no K projection) and scale by D**-0.5.

Sharding: 8 cores = 4 batch elements x 2 sequence halves. Each core owns
512 query rows of one batch element. Per layer, each core computes its half's
V projection (token-major, bf16) and the pair exchanges halves via a 2-rank
AllGather, so every core holds all 1024 keys/values of its batch element.
Everything else (LN, Q, scores, softmax, ctx, Wo, FFN) is computed per-core
on its own 512 rows.

Rev1 notes:
- LayerNorm rsqrt computed on DVE (Quake bit-trick + 2 Newton steps),
  batched over the 4 token tiles: no ACT table switches for LN.
- h token-major -> feature-major transposes via DMA xbar transpose
  (SBUF->SBUF), off the PE.
- Scores pack both heads of a head-pair into one N=1024 matmul against a
  zero-padded q tile.
- Softmax Z: reciprocal_approx_fast + K=1 PE matmul broadcast (no gpsimd).
"""
import os
import sys

sys.path.insert(0, "/opt/trn_rl_repo")

import numpy as np
import ml_dtypes

import concourse.bass as bass
from concourse.bass import ds
from concourse import bacc
import concourse.tile as tile
from concourse import mybir
from concourse.bass_utils import run_bass_kernel_spmd

P = 128
D = 768
H = 12
DH = 64
FF = 3072
NB_D = 6          # D / P
NB_T = 4          # own tokens 512 / P
NB_K = 8          # full tokens 1024 / P
NB_FF = 24        # FF / P
T_OWN = 512
SCALE = float(D) ** -0.5
LN_EPS = 1e-5
N_LAYERS = int(os.environ.get("KERNEL_N_LAYERS", "12"))
LN_ACT = int(os.environ.get("KERNEL_LN_ACT", "0"))
SLOW_RECIP = int(os.environ.get("KERNEL_SLOW_RECIP", "0"))

F32 = mybir.dt.float32
F32R = mybir.dt.float32r
I32 = mybir.dt.int32
BF16 = mybir.dt.bfloat16
AF = mybir.ActivationFunctionType
OP = mybir.AluOpType

REPLICA_GROUPS = [[0, 1], [2, 3], [4, 5], [6, 7]]

_cached = {}
_last_results = None


def _register_ntff_hook():
    """Register the axon NTFF profile hook (for trace=True exec timing)."""
    import types
    try:
        import antenv.axon_hooks  # noqa: F401
        return
    except ImportError:
        pass
    try:
        from trn_agent_boot.trn_boot import _ntff_profile_via_ctypes
        import antenv
        hook = _ntff_profile_via_ctypes("/opt/axon/libaxon_pjrt.so")
        mod = types.ModuleType("antenv.axon_hooks")
        mod.get_axon_ntff_profile_hook = lambda: hook
        mod.set_axon_ntff_profile_hook = lambda h: None
        sys.modules["antenv.axon_hooks"] = mod
        antenv.axon_hooks = mod
    except Exception:
        pass


def _regions():
    return ((0, 512), (512, 768))


def build(n_layers=N_LAYERS):
    nc = bacc.Bacc(None, target_bir_lowering=False, num_devices=8)
    L = n_layers

    x_d = nc.dram_tensor("x", [P, NB_T, D], F32R, kind="ExternalInput")
    wq_d = nc.dram_tensor("wq", [L, P, NB_D * D], BF16, kind="ExternalInput")
    wv_d = nc.dram_tensor("wv", [L, P, NB_D * D], BF16, kind="ExternalInput")
    wo_d = nc.dram_tensor("wo", [L, P, NB_D * D], BF16, kind="ExternalInput")
    w1_d = nc.dram_tensor("w1", [L, 4, P, NB_D * D], BF16, kind="ExternalInput")
    w2_d = nc.dram_tensor("w2", [L, 4, P, NB_D * D], BF16, kind="ExternalInput")
    bq_d = nc.dram_tensor("bq", [P, L, NB_D], F32, kind="ExternalInput")
    b1_d = nc.dram_tensor("b1", [P, L, NB_FF], F32, kind="ExternalInput")
    bvr_d = nc.dram_tensor("bv_row", [1, L, D], F32, kind="ExternalInput")
    bor_d = nc.dram_tensor("bo_row", [1, L, D], BF16, kind="ExternalInput")
    b2r_d = nc.dram_tensor("b2_row", [1, L, D], BF16, kind="ExternalInput")
    idbf_d = nc.dram_tensor("identbf", [P, P], BF16, kind="ExternalInput")
    ones_d = nc.dram_tensor("ones1", [1, P], BF16, kind="ExternalInput")
    out_d = nc.dram_tensor("out", [P, NB_T, D], F32R, kind="ExternalOutput")

    with tile.TileContext(nc) as tc:
        with (
            tc.tile_pool(name="state", bufs=1) as st,
            tc.tile_pool(name="stream", bufs=2) as sp,
            tc.tile_pool(name="acts", bufs=1) as ap,
            tc.tile_pool(name="wpool", bufs=2) as wp,
            tc.tile_pool(name="psA", bufs=2, space="PSUM") as psA,
            tc.tile_pool(name="psB", bufs=4, space="PSUM") as psB,
            tc.tile_pool(name="dram", bufs=2, space="DRAM") as dp,
        ):
            # ---- constants ----
            idbf = st.tile([P, P], BF16)
            ones1 = st.tile([1, P], BF16)
            magic = st.tile([P, NB_T], I32)
            bq_all = st.tile([P, L, NB_D], F32)
            b1_all = st.tile([P, L, NB_FF], F32)
            q_par = st.tile([P, NB_D, T_OWN], BF16)
            v_aug = st.tile([P, NB_K, H * 65], BF16)
            va4 = v_aug.rearrange("p k (h c) -> p k h c", c=65)
            nc.vector.memset(magic[:], 0x5F3759DF)
            for kb in range(NB_K):
                nc.vector.memset(va4[:, kb, :, 64:65], 1.0)
            nc.sync.dma_start(idbf[:], idbf_d[:])
            nc.sync.dma_start(ones1[:], ones_d[:])
            nc.sync.dma_start(bq_all[:], bq_d[:])
            nc.sync.dma_start(b1_all[:], b1_d[:])

            eps_t = st.tile([P, 1], F32)
            nc.vector.memset(eps_t[:], LN_EPS)
            dummy = st.tile([P, 1], F32)
            peer_off = (1 - (nc.gpsimd.partition_id() % 2)) * NB_T

            def ln_alloc():
                stt = ap.tile([P, NB_T, 3, 6], F32, tag="lnstat", bufs=2, name="lnst")
                mv = ap.tile([P, NB_T, 2], F32, tag="lnmv", bufs=2, name="lnmv")
                return stt, mv

            def ln_stats(stt, mv, tb, x_in):
                xg = x_in[:, tb, :].rearrange("p (g d) -> p g d", g=3)
                for g in range(3):
                    nc.vector.bn_stats(stt[:, tb, g, :], xg[:, g, :])
                nc.vector.bn_aggr(mv[:, tb, :], stt[:, tb, :, :])

            def ln_finish(h_out, x_in, stt, mv):
                rs = ap.tile([P, NB_T], F32, tag="lnrs", bufs=2, name="lnrs")
                tmp = ap.tile([P, 2, NB_T], F32, tag="lntmp", bufs=2, name="lntmp")
                var = mv[:, :, 1:2].rearrange("p a b -> p (a b)")
                y = tmp[:, 0, :]
                nc.vector.tensor_scalar(
                    out=rs[:], in0=var, scalar1=LN_EPS, scalar2=None, op0=OP.add,
                )
                s_i = rs[:].bitcast(I32)
                y_i = tmp[:, 0, :].bitcast(I32)
                nc.vector.tensor_scalar(
                    out=y_i, in0=s_i, scalar1=1, scalar2=None,
                    op0=OP.logical_shift_right,
                )
                nc.vector.tensor_tensor(y_i, magic[:], y_i, OP.subtract)
                t2 = tmp[:, 1, :]
                for _ in range(2):
                    nc.vector.tensor_tensor(t2, y, y, OP.mult)
                    nc.vector.tensor_tensor(t2, t2, rs[:], OP.mult)
                    nc.vector.tensor_scalar(
                        out=t2, in0=t2, scalar1=-0.5, scalar2=1.5,
                        op0=OP.mult, op1=OP.add,
                    )
                    nc.vector.tensor_tensor(y, y, t2, OP.mult)
                for tb in range(NB_T):
                    nc.vector.tensor_scalar(
                        out=h_out[:, tb, :], in0=x_in[:, tb, :],
                        scalar1=mv[:, tb, 0:1], scalar2=y[:, tb:tb + 1],
                        op0=OP.subtract, op1=OP.mult,
                    )

            def ln_group(h_out, x_in):
                """LayerNorm (normalize only) of 4 [128, 768] tiles.

                Stats via bn_stats/bn_aggr; rsqrt(var+eps) via the fp32
                bit-trick seed + 2 Newton steps, all on DVE (no ACT) --
                unless KERNEL_LN_ACT, then via ACT Ln/Exp as in baseline.
                h_out is bf16 [P, NB_T, D]; x_in is f32r [P, NB_T, D].
                """
                stt = ap.tile([P, NB_T, 3, 6], F32, tag="lnstat", bufs=2, name="lnst")
                mv = ap.tile([P, NB_T, 2], F32, tag="lnmv", bufs=2, name="lnmv")
                rs = ap.tile([P, NB_T], F32, tag="lnrs", bufs=2, name="lnrs")
                tmp = ap.tile([P, 2, NB_T], F32, tag="lntmp", bufs=2, name="lntmp")
                for tb in range(NB_T):
                    xg = x_in[:, tb, :].rearrange("p (g d) -> p g d", g=3)
                    for g in range(3):
                        nc.vector.bn_stats(stt[:, tb, g, :], xg[:, g, :])
                    nc.vector.bn_aggr(mv[:, tb, :], stt[:, tb, :, :])
                var = mv[:, :, 1:2].rearrange("p a b -> p (a b)")
                y = tmp[:, 0, :]
                if LN_ACT:
                    nc.scalar.activation(tmp[:, 1, :], var, AF.Ln, bias=eps_t[:], scale=1.0)
                    nc.scalar.activation(y, tmp[:, 1, :], AF.Exp, scale=-0.5)
                else:
                    # s = var + eps (f32), y0 = bits(0x5f3759df - (s_int >> 1))
                    nc.vector.tensor_scalar(
                        out=rs[:], in0=var, scalar1=LN_EPS, scalar2=None, op0=OP.add,
                    )
                    s_i = rs[:].bitcast(I32)
                    y_i = tmp[:, 0, :].bitcast(I32)
                    nc.vector.tensor_scalar(
                        out=y_i, in0=s_i, scalar1=1, scalar2=None,
                        op0=OP.logical_shift_right,
                    )
                    nc.vector.tensor_tensor(y_i, magic[:], y_i, OP.subtract)
                    t2 = tmp[:, 1, :]
                    for _ in range(2):
                        nc.vector.tensor_tensor(t2, y, y, OP.mult)
                        nc.vector.tensor_tensor(t2, t2, rs[:], OP.mult)
                        nc.vector.tensor_scalar(
                            out=t2, in0=t2, scalar1=-0.5, scalar2=1.5,
                            op0=OP.mult, op1=OP.add,
                        )
                        nc.vector.tensor_tensor(y, y, t2, OP.mult)
                for tb in range(NB_T):
                    nc.vector.tensor_scalar(
                        out=h_out[:, tb, :], in0=x_in[:, tb, :],
                        scalar1=mv[:, tb, 0:1], scalar2=y[:, tb:tb + 1],
                        op0=OP.subtract, op1=OP.mult,
                    )

            def transpose_tm_to_fm(h_fm, h_tm, h_dr, eng):
                """[128, 4, 768] bf16 token-major -> [128, 6, 512] bf16
                feature-major via a DRAM store + DMA xbar transposes."""
                for tb in range(NB_T):
                    eng.dma_start(h_dr[tb], h_tm[:, tb, :])
                ht_flat = h_dr.rearrange("k p n -> (k p) n")
                for db in range(NB_D):
                    eng.dma_start_transpose(
                        h_fm[:, db, :], ht_flat[:, db * P:(db + 1) * P]
                    )

            def transpose_tm_to_fm_pe(h_fm, h_tm, tagsfx):
                """Same transform on the PE (bf16), for when the PE would
                otherwise idle waiting on the DMA chain."""
                for db in range(NB_D):
                    trp = psB.tile([P, T_OWN], BF16, tag="psB", name=f"trp{tagsfx}_{db}")
                    for tb in range(NB_T):
                        nc.tensor.transpose(
                            trp[:, tb * P:(tb + 1) * P],
                            h_tm[:, tb, db * P:(db + 1) * P], idbf[:],
                        )
                    nc.vector.tensor_copy(h_fm[:, db, :], trp[:])

            # ---- initial stream ----
            x_t = sp.tile([P, NB_T, D], F32R, tag="stream", name="x0")
            nc.sync.dma_start(x_t[:], x_d[:])

            for l in range(L):
                # ---- weights for this layer ----
                wq = wp.tile([P, NB_D, D], BF16, tag="wbf", bufs=5, name=f"wq{l}")
                nc.sync.dma_start(wq[:], wq_d[l].rearrange("p (k n) -> p k n", n=D))
                wv = wp.tile([P, NB_D, D], BF16, tag="wbf", bufs=5, name=f"wv{l}")
                nc.sync.dma_start(wv[:], wv_d[l].rearrange("p (k n) -> p k n", n=D))
                bv_row = ap.tile([1, D], F32, tag="bvrow", bufs=2, name=f"bvr{l}")
                nc.sync.dma_start(bv_row[:], bvr_d[:, l, :])
                bo_row = ap.tile([1, D], BF16, tag="borow", bufs=2, name=f"bor{l}")
                nc.sync.dma_start(bo_row[:], bor_d[:, l, :])
                b2_row = ap.tile([1, D], BF16, tag="b2row", bufs=2, name=f"b2r{l}")
                nc.sync.dma_start(b2_row[:], b2r_d[:, l, :])

                # ---- LN1 + transpose ----
                with nc.named_scope(f"L{l:02d}_a_ln1"):
                    h_tm = ap.tile([P, NB_T, D], BF16, tag="h_tm", bufs=1, name=f"h1tm{l}")
                    ln_group(h_tm, x_t)
                    h1_fm = ap.tile([P, NB_D, T_OWN], BF16, tag="h_fm", bufs=1,
                                    name=f"h1fm{l}")
                    transpose_tm_to_fm_pe(h1_fm, h_tm, f"h1_{l}")

                # ---- V token-major (+bias) -> send buffer ----
                nc.enter_named_scope(f"L{l:02d}_b_v", False)
                bv_bc = ap.tile([P, D], F32, tag="bv_bc", bufs=1, name=f"bvbc{l}")
                nc.gpsimd.partition_broadcast(bv_bc[:], bv_row[:])
                v_send = ap.tile([P, NB_T, D], BF16, tag="vsend", bufs=1, name=f"vsend{l}")
                for tb in range(NB_T):
                    vp = psA.tile([P, D], F32, tag="psA", name=f"vps{l}_{tb}")
                    for n0, n1 in _regions():
                        for kb in range(NB_D):
                            nc.tensor.matmul(
                                vp[:, n0:n1],
                                h1_fm[:, kb, tb * P:(tb + 1) * P],
                                wv[:, kb, n0:n1],
                                start=(kb == 0), stop=(kb == NB_D - 1),
                            )
                    nc.vector.tensor_tensor(v_send[:, tb, :], vp[:], bv_bc[:], OP.add)

                nc.leave_named_scope(f"L{l:02d}_b_v", None, False)
                nc.enter_named_scope(f"L{l:02d}_c_ag", False)
                vsend_dr = dp.tile([NB_T, P, D], BF16, tag="vsend_d", name=f"vsdr{l}")
                for tb in range(NB_T):
                    nc.sync.dma_start(vsend_dr[tb], v_send[:, tb, :])
                vrecv_dr = dp.tile([NB_K, P, D], BF16, tag="vrecv_d", name=f"vrdr{l}")
                nc.gpsimd.collective_compute(
                    "AllGather", OP.bypass, replica_groups=REPLICA_GROUPS,
                    ins=[vsend_dr[:]], outs=[vrecv_dr[:]],
                )
                nc.leave_named_scope(f"L{l:02d}_c_ag", None, False)
                nc.enter_named_scope(f"L{l:02d}_d_q", False)
                # ---- Q (overlaps the AllGather) ----
                for m in range(NB_D):
                    qp = psB.tile([P, T_OWN], F32, tag="psB", name=f"qps{l}_{m}")
                    for kb in range(NB_D):
                        nc.tensor.matmul(
                            qp[:], wq[:, kb, m * P:(m + 1) * P], h1_fm[:, kb, :],
                            start=(kb == 0), stop=(kb == NB_D - 1),
                        )
                    nc.vector.tensor_scalar(
                        out=q_par[:, m, :], in0=qp[:],
                        scalar1=bq_all[:, l, m:m + 1], scalar2=None,
                        op0=OP.add,
                    )

                # prefetch Wo while attention runs
                wo = wp.tile([P, NB_D, D], BF16, tag="wbf", bufs=5, name=f"wo{l}")
                nc.sync.dma_start(wo[:], wo_d[l].rearrange("p (k n) -> p k n", n=D))

                nc.leave_named_scope(f"L{l:02d}_d_q", None, False)
                nc.enter_named_scope(f"L{l:02d}_e_vrecv", False)
                # ---- receive V: augmented token-major + feature-major ----
                # own-relative key order: kb 0-3 = own half (local data),
                # kb 4-7 = peer half (dynamic shard pick from vrecv)
                v_fm = ap.tile([P, NB_D, NB_K * P], BF16, tag="v_fm", bufs=1, name=f"vfm{l}")
                for tb in range(NB_T):
                    nc.vector.tensor_copy(
                        va4[:, tb, :, 0:64],
                        v_send[:, tb, :].rearrange("p (h c) -> p h c", c=64),
                    )
                # own-half v_fm via PE transposes straight from SBUF: any DMA
                # here queues behind the AllGather's transfers and stalls attention
                for db in range(NB_D):
                    vto = psA.tile([P, T_OWN], BF16, tag="psA", name=f"vto{l}_{db}")
                    for tb in range(NB_T):
                        nc.tensor.transpose(
                            vto[:, tb * P:(tb + 1) * P],
                            v_send[:, tb, db * P:(db + 1) * P], idbf[:],
                        )
                    nc.vector.tensor_copy(v_fm[:, db, 0:T_OWN], vto[:])
                nc.gpsimd.dma_start(
                    va4[:, 4:8, :, 0:64],
                    vrecv_dr.rearrange("k p (h c) -> k p h c", c=64)[ds(peer_off, 4)],
                )
                nc.leave_named_scope(f"L{l:02d}_e_vrecv", None, False)
                nc.enter_named_scope(f"L{l:02d}_f_attn", False)
                # ---- attention: flat software pipeline over (db, kb) steps;
                # ctx lags scores by one step; db 0-1 own-half steps overlap
                # the AllGather ----
                ctx_n = ap.tile([P, NB_D, T_OWN], BF16, tag="ctx_n", bufs=1, name=f"ctxn{l}")
                # step schedule: (db, kb, is_first_of_db, is_last_of_db)
                steps = []
                for db in (0, 1):
                    for kb in (0, 1, 2, 3):
                        steps.append((db, kb, kb == 0, False))
                steps.append("PEER_TR")
                for db in (0, 1):
                    for kb in (4, 5, 6, 7):
                        steps.append((db, kb, False, kb == 7))
                for db in range(2, NB_D):
                    for kb in range(NB_K):
                        steps.append((db, kb, kb == 0, kb == NB_K - 1))

                ctxps = {}
                pending = None  # (db, kb, ex, is_first, is_last)

                def flush_ctx():
                    nonlocal pending
                    if pending is None:
                        return
                    db, kb, ex, first, last = pending
                    hpair = (2 * db, 2 * db + 1)
                    for hh in range(2):
                        nc.tensor.matmul(
                            ctxps[db][hh][:],
                            v_aug[:, kb, 65 * hpair[hh]:65 * hpair[hh] + 65],
                            ex[:, 512 * hh:512 * hh + 512],
                            start=first, stop=last,
                        )
                    if last:
                        for hh in range(2):
                            h = hpair[hh]
                            r0 = 64 * hh
                            zsc = ap.tile([1, T_OWN], F32, tag="zsc", bufs=2,
                                          name=f"zs{l}_{h}")
                            if SLOW_RECIP:
                                nc.vector.reciprocal(zsc[:], ctxps[db][hh][64:65, :])
                            else:
                                zrow = ap.tile([1, T_OWN], F32, tag="zrow", bufs=2,
                                               name=f"zr{l}_{h}")
                                nc.vector.tensor_copy(zrow[:], ctxps[db][hh][64:65, :])
                                nc.vector.reciprocal_approx_fast(
                                    out=zsc[:], in_=zrow[:],
                                )
                            zbc = ap.tile([P, T_OWN], F32, tag="zbc", bufs=2,
                                          name=f"zb{l}_{h}")
                            nc.gpsimd.partition_broadcast(zbc[:], zsc[:])
                            nc.vector.tensor_tensor(
                                ctx_n[r0:r0 + 64, db, :], ctxps[db][hh][0:64, :],
                                zbc[0:64, :], OP.mult,
                            )
                        del ctxps[db]
                    pending = None

                for stp in steps:
                    if stp == "PEER_TR":
                        # peer half of v_fm via PE transposes of own-relative v_aug
                        for db in range(NB_D):
                            vtp = psA.tile([P, T_OWN], BF16, tag="psA",
                                           name=f"vtp{l}_{db}")
                            for tb in range(4, NB_K):
                                for hh in range(2):
                                    nc.tensor.transpose(
                                        vtp[64 * hh:64 * hh + 64,
                                            (tb - 4) * P:(tb - 3) * P],
                                        v_aug[:, tb,
                                              65 * (2 * db + hh):65 * (2 * db + hh) + 64],
                                        idbf[:],
                                    )
                            nc.vector.tensor_copy(v_fm[:, db, T_OWN:], vtp[:])
                        continue
                    db, kb, first, last = stp
                    if first:
                        ctxps[db] = [
                            psB.tile([65, T_OWN], F32, tag="psB",
                                     name=f"ctxp{l}_{2 * db + hh}")
                            for hh in range(2)
                        ]
                    spv = psA.tile([P, 2 * T_OWN], F32, tag="psA",
                                   name=f"sc{l}_{db}_{kb}")
                    for hh in range(2):
                        r0 = 64 * hh
                        nc.tensor.matmul(
                            spv[:, 512 * hh:512 * hh + 512],
                            v_fm[r0:r0 + 64, db, kb * P:(kb + 1) * P],
                            q_par[r0:r0 + 64, db, :],
                            start=True, stop=True,
                        )
                    flush_ctx()
                    ex = ap.tile([P, 2 * T_OWN], BF16, tag="expT", bufs=4,
                                 name=f"ex{l}_{db}_{kb}")
                    nc.scalar.activation(ex[:], spv[:], AF.Exp, scale=SCALE)
                    pending = (db, kb, ex, first, last)
                flush_ctx()
                nc.leave_named_scope(f"L{l:02d}_f_attn", None, False)
                nc.enter_named_scope(f"L{l:02d}_g_wo", False)
                # ---- Wo + residual (+ LN2 stats per block as it completes) ----
                skip = sp.tile([P, NB_T, D], F32R, tag="stream", name=f"skip{l}")
                h_tm2 = ap.tile([P, NB_T, D], BF16, tag="h_tm2", bufs=1, name=f"h2tm{l}")
                stt2, mv2 = ln_alloc()
                for lb in range(NB_T):
                    wps = psA.tile([P, D], F32, tag="psA", name=f"wops{l}_{lb}")
                    for n0, n1 in _regions():
                        for kb in range(NB_D):
                            nc.tensor.matmul(
                                wps[:, n0:n1],
                                ctx_n[:, kb, lb * P:(lb + 1) * P],
                                wo[:, kb, n0:n1],
                                start=(kb == 0), stop=False,
                            )
                        nc.tensor.matmul(
                            wps[:, n0:n1], ones1[:], bo_row[:, n0:n1],
                            start=False, stop=True,
                        )
                    nc.vector.tensor_tensor(skip[:, lb, :], x_t[:, lb, :], wps[:], OP.add)
                    ln_stats(stt2, mv2, lb, skip)
                nc.leave_named_scope(f"L{l:02d}_g_wo", None, False)
                nc.enter_named_scope(f"L{l:02d}_h_ln2", False)
                # ---- LN2 + transpose ----
                ln_finish(h_tm2, skip, stt2, mv2)
                h2_fm = ap.tile([P, NB_D, T_OWN], BF16, tag="h_fm2", bufs=1,
                                name=f"h2fm{l}")
                transpose_tm_to_fm_pe(h2_fm, h_tm2, f"h2_{l}")
                nc.leave_named_scope(f"L{l:02d}_h_ln2", None, False)
                nc.enter_named_scope(f"L{l:02d}_i_ff", False)
                # ---- FFN: FF1 (all 24 hidden blocks) then FF2 accumulated in PSUM ----
                g_all = ap.tile([P, NB_FF, T_OWN], BF16, tag="g", bufs=1, name=f"g{l}")
                w2cs = []
                for ck in range(4):
                    w1c = wp.tile([P, NB_D, D], BF16, tag="wbf", bufs=5, name=f"w1c{l}_{ck}")
                    nc.sync.dma_start(w1c[:], w1_d[l, ck].rearrange("p (k n) -> p k n", n=D))
                    w2c = wp.tile([P, NB_D, D], BF16, tag="wbf", bufs=5, name=f"w2c{l}_{ck}")
                    nc.sync.dma_start(w2c[:], w2_d[l, ck].rearrange("p (k n) -> p k n", n=D))
                    w2cs.append(w2c)
                    for mm in range(NB_D):
                        fp = psB.tile([P, T_OWN], F32, tag="psB", name=f"f1ps{l}_{ck}_{mm}")
                        for kb in range(NB_D):
                            nc.tensor.matmul(
                                fp[:], w1c[:, kb, mm * P:(mm + 1) * P], h2_fm[:, kb, :],
                                start=(kb == 0), stop=(kb == NB_D - 1),
                            )
                        nc.scalar.activation(
                            g_all[:, 6 * ck + mm, :], fp[:], AF.Gelu,
                            bias=b1_all[:, l, 6 * ck + mm:6 * ck + mm + 1], scale=1.0,
                        )
                nc.scalar.activation(dummy[:], eps_t[:], AF.Exp, scale=1.0)
                for half in range(2):
                    f2s = []
                    for lb in (2 * half, 2 * half + 1):
                        f2 = psA.tile([P, D], F32, tag="psA", name=f"f2ps{l}_{lb}")
                        f2s.append(f2)
                        for n0, n1 in _regions():
                            for ck in range(4):
                                for mm in range(NB_D):
                                    nc.tensor.matmul(
                                        f2[:, n0:n1],
                                        g_all[:, 6 * ck + mm, lb * P:(lb + 1) * P],
                                        w2cs[ck][:, mm, n0:n1],
                                        start=(ck == 0 and mm == 0), stop=False,
                                    )
                            nc.tensor.matmul(
                                f2[:, n0:n1], ones1[:], b2_row[:, n0:n1],
                                start=False, stop=True,
                            )
                    for i, lb in enumerate((2 * half, 2 * half + 1)):
                        nc.vector.tensor_tensor(
                            skip[:, lb, :], skip[:, lb, :], f2s[i][:], OP.add,
                        )
                nc.leave_named_scope(f"L{l:02d}_i_ff", None, False)
                x_t = skip

            nc.sync.dma_start(out_d[:], x_t[:])
    nc.compile()
    return nc


def _preprocess(inputs, n_layers):
    """Fold LN affine into projections; lay out weights for tile DMA."""
    f32 = np.float32
    L = n_layers
    Wq = np.asarray(inputs["Wq"], f32)[:L]
    Wv = np.asarray(inputs["Wv"], f32)[:L]
    Wo = np.asarray(inputs["Wo"], f32)[:L]
    W1 = np.asarray(inputs["W1"], f32)[:L]
    W2 = np.asarray(inputs["W2"], f32)[:L]
    g1 = np.asarray(inputs["ln1_g"], f32)[:L]
    b1ln = np.asarray(inputs["ln1_b"], f32)[:L]
    g2 = np.asarray(inputs["ln2_g"], f32)[:L]
    b2ln = np.asarray(inputs["ln2_b"], f32)[:L]
    bq = np.asarray(inputs["bq"], f32)[:L]
    bv = np.asarray(inputs["bv"], f32)[:L]
    bo = np.asarray(inputs["bo"], f32)[:L]
    b1 = np.asarray(inputs["b1"], f32)[:L]
    b2 = np.asarray(inputs["b2"], f32)[:L]

    Wq_eff = g1[:, :, None] * Wq
    bq_eff = bq + np.einsum("ld,ldo->lo", b1ln, Wq)
    Wv_eff = g1[:, :, None] * Wv
    bv_eff = bv + np.einsum("ld,ldo->lo", b1ln, Wv)
    W1_eff = g2[:, :, None] * W1
    b1_eff = b1 + np.einsum("ld,ldo->lo", b2ln, W1)

    def fm_weight(W):  # [L, D, D] -> [L, 128, 6*768] with [p, k, n]
        return np.ascontiguousarray(
            W.reshape(L, NB_D, P, D).transpose(0, 2, 1, 3).reshape(L, P, NB_D * D)
        )

    bf = ml_dtypes.bfloat16
    wq_h = fm_weight(Wq_eff).astype(bf)
    wv_h = fm_weight(Wv_eff).astype(bf)
    wo_h = fm_weight(Wo).astype(bf)
    w1_h = np.ascontiguousarray(
        W1_eff.reshape(L, NB_D, P, 4, D).transpose(0, 3, 2, 1, 4).reshape(L, 4, P, NB_D * D)
    ).astype(bf)
    w2_h = np.ascontiguousarray(
        W2.reshape(L, 4, NB_D, P, D).transpose(0, 1, 3, 2, 4).reshape(L, 4, P, NB_D * D)
    ).astype(ml_dtypes.bfloat16)
    bq_h = np.ascontiguousarray(bq_eff.reshape(L, NB_D, P).transpose(2, 0, 1))
    b1_h = np.ascontiguousarray(b1_eff.reshape(L, NB_FF, P).transpose(2, 0, 1))

    return {
        "wq": wq_h, "wv": wv_h, "wo": wo_h, "w1": w1_h, "w2": w2_h,
        "bq": bq_h, "b1": b1_h,
        "bv_row": np.ascontiguousarray(bv_eff[None]),
        "bo_row": np.ascontiguousarray(bo[None]).astype(bf),
        "b2_row": np.ascontiguousarray(b2[None]).astype(bf),
        "identbf": np.eye(P).astype(ml_dtypes.bfloat16),
        "ones1": np.ones((1, P)).astype(bf),
    }


def kernel(**inputs) -> np.ndarray:
    n_layers = N_LAYERS
    key = ("nc", n_layers)
    if key not in _cached:
        _cached[key] = build(n_layers)
    nc = _cached[key]

    shared = _preprocess(inputs, n_layers)
    x = np.asarray(inputs["x"], np.float32)  # [4, 1024, 768]
    B, T, _ = x.shape

    in_maps = []
    for c in range(8):
        b, half = c // 2, c % 2
        x_own = x[b, half * T_OWN:(half + 1) * T_OWN]          # [512, 768]
        x_tile = np.ascontiguousarray(
            x_own.reshape(NB_T, P, D).transpose(1, 0, 2)        # [128, 4, 768]
        )
        in_maps.append({**shared, "x": x_tile})

    trace = bool(int(os.environ.get("KERNEL_TRACE", "0")))
    if trace:
        _register_ntff_hook()
    res = run_bass_kernel_spmd(nc, in_maps, core_ids=list(range(8)), trace=trace)
    global _last_results
    _last_results = res

    out = np.empty((B, T, D), dtype=np.float32)
    for c in range(8):
        b, half = c // 2, c % 2
        o = res.results[c]["out"]                               # [128, 4, 768]
        out[b, half * T_OWN:(half + 1) * T_OWN] = (
            o.transpose(1, 0, 2).reshape(T_OWN, D)
        )
    return out


# revision 22
# speedup vs baseline: 1.0695x; 1.0695x over previous
"""Trainium2 Bass kernel for nn_Encoder_506806141403.

12-layer transformer encoder (D=768, H=12, FF=3072) with the quirk that
attention scores use Q vs V (no K projection) and scale by D**-0.5.

Sharding: 8 cores = 4 batch elements x 2 sequence halves. Each core owns
512 query rows of one batch element. Per layer, each core computes its half's
V projection (token-major, bf16) and the pair exchanges halves via a 2-rank
AllGather, so every core holds all 1024 keys/values of its batch element.
Everything else (LN, Q, scores, softmax, ctx, Wo, FFN) is computed per-core
on its own 512 rows.

Rev1 notes:
- LayerNorm rsqrt computed on DVE (Quake bit-trick + 2 Newton steps),
  batched over the 4 token tiles: no ACT table switches for LN.
- h token-major -> feature-major transposes via DMA xbar transpose
  (SBUF->SBUF), off the PE.
- Scores pack both heads of a head-pair into one N=1024 matmul against a
  zero-padded q tile.
- Softmax Z: reciprocal_approx_fast + K=1 PE matmul broadcast (no gpsimd).
"""
import os
import sys

sys.path.insert(0, "/opt/trn_rl_repo")

import numpy as np
import ml_dtypes

import concourse.bass as bass
from concourse.bass import ds
from concourse import bacc
import concourse.tile as tile
from concourse import mybir
from concourse.bass_utils import run_bass_kernel_spmd

P = 128
D = 768
H = 12
DH = 64
FF = 3072
NB_D = 6          # D / P
NB_T = 4          # own tokens 512 / P
NB_K = 8          # full tokens 1024 / P
NB_FF = 24        # FF / P
T_OWN = 512
SCALE = float(D) ** -0.5
LN_EPS = 1e-5
N_LAYERS = int(os.environ.get("KERNEL_N_LAYERS", "12"))
LN_ACT = int(os.environ.get("KERNEL_LN_ACT", "0"))
SLOW_RECIP = int(os.environ.get("KERNEL_SLOW_RECIP", "0"))

F32 = mybir.dt.float32
F32R = mybir.dt.float32r
I32 = mybir.dt.int32
BF16 = mybir.dt.bfloat16
AF = mybir.ActivationFunctionType
OP = mybir.AluOpType

REPLICA_GROUPS = [[0, 1], [2, 3], [4, 5], [6, 7]]

_cached = {}
_last_results = None


def _register_ntff_hook():
    """Register the axon NTFF profile hook (for trace=True exec timing)."""
    import types
    try:
        import antenv.axon_hooks  # noqa: F401
        return
    except ImportError:
        pass
    try:
        from trn_agent_boot.trn_boot import _ntff_profile_via_ctypes
        import antenv
        hook = _ntff_profile_via_ctypes("/opt/axon/libaxon_pjrt.so")
        mod = types.ModuleType("antenv.axon_hooks")
        mod.get_axon_ntff_profile_hook = lambda: hook
        mod.set_axon_ntff_profile_hook = lambda h: None
        sys.modules["antenv.axon_hooks"] = mod
        antenv.axon_hooks = mod
    except Exception:
        pass


def _regions():
    return ((0, 512), (512, 768))


def build(n_layers=N_LAYERS):
    nc = bacc.Bacc(None, target_bir_lowering=False, num_devices=8)
    L = n_layers

    x_d = nc.dram_tensor("x", [P, NB_T, D], F32R, kind="ExternalInput")
    wq_d = nc.dram_tensor("wq", [L, P, NB_D * D], BF16, kind="ExternalInput")
    wv_d = nc.dram_tensor("wv", [L, P, NB_D * D], BF16, kind="ExternalInput")
    wo_d = nc.dram_tensor("wo", [L, P, NB_D * D], BF16, kind="ExternalInput")
    w1_d = nc.dram_tensor("w1", [L, 4, P, NB_D * D], BF16, kind="ExternalInput")
    w2_d = nc.dram_tensor("w2", [L, 4, P, NB_D * D], BF16, kind="ExternalInput")
    bq_d = nc.dram_tensor("bq", [P, L, NB_D], F32, kind="ExternalInput")
    b1_d = nc.dram_tensor("b1", [P, L, NB_FF], F32, kind="ExternalInput")
    bvr_d = nc.dram_tensor("bv_row", [1, L, D], F32, kind="ExternalInput")
    bor_d = nc.dram_tensor("bo_row", [1, L, D], BF16, kind="ExternalInput")
    b2r_d = nc.dram_tensor("b2_row", [1, L, D], BF16, kind="ExternalInput")
    idbf_d = nc.dram_tensor("identbf", [P, P], BF16, kind="ExternalInput")
    ones_d = nc.dram_tensor("ones1", [1, P], BF16, kind="ExternalInput")
    out_d = nc.dram_tensor("out", [P, NB_T, D], F32R, kind="ExternalOutput")

    with tile.TileContext(nc) as tc:
        with (
            tc.tile_pool(name="state", bufs=1) as st,
            tc.tile_pool(name="stream", bufs=2) as sp,
            tc.tile_pool(name="acts", bufs=1) as ap,
            tc.tile_pool(name="wpool", bufs=2) as wp,
            tc.tile_pool(name="psA", bufs=2, space="PSUM") as psA,
            tc.tile_pool(name="psB", bufs=4, space="PSUM") as psB,
            tc.tile_pool(name="dram", bufs=2, space="DRAM") as dp,
        ):
            # ---- constants ----
            idbf = st.tile([P, P], BF16)
            ones1 = st.tile([1, P], BF16)
            magic = st.tile([P, NB_T], I32)
            bq_all = st.tile([P, L, NB_D], F32)
            b1_all = st.tile([P, L, NB_FF], F32)
            q_par = st.tile([P, NB_D, T_OWN], BF16)
            v_aug = st.tile([P, NB_K, H * 65], BF16)
            va4 = v_aug.rearrange("p k (h c) -> p k h c", c=65)
            nc.vector.memset(magic[:], 0x5F3759DF)
            for kb in range(NB_K):
                nc.vector.memset(va4[:, kb, :, 64:65], 1.0)
            nc.sync.dma_start(idbf[:], idbf_d[:])
            nc.sync.dma_start(ones1[:], ones_d[:])
            nc.sync.dma_start(bq_all[:], bq_d[:])
            nc.sync.dma_start(b1_all[:], b1_d[:])

            eps_t = st.tile([P, 1], F32)
            nc.vector.memset(eps_t[:], LN_EPS)

            def ln_group(h_out, x_in):
                """LayerNorm (normalize only) of 4 [128, 768] tiles.

                Stats via bn_stats/bn_aggr; rsqrt(var+eps) via the fp32
                bit-trick seed + 2 Newton steps, all on DVE (no ACT) --
                unless KERNEL_LN_ACT, then via ACT Ln/Exp as in baseline.
                h_out is bf16 [P, NB_T, D]; x_in is f32r [P, NB_T, D].
                """
                stt = ap.tile([P, NB_T, 3, 6], F32, tag="lnstat", bufs=2, name="lnst")
                mv = ap.tile([P, NB_T, 2], F32, tag="lnmv", bufs=2, name="lnmv")
                rs = ap.tile([P, NB_T], F32, tag="lnrs", bufs=2, name="lnrs")
                tmp = ap.tile([P, 2, NB_T], F32, tag="lntmp", bufs=2, name="lntmp")
                for tb in range(NB_T):
                    xg = x_in[:, tb, :].rearrange("p (g d) -> p g d", g=3)
                    for g in range(3):
                        nc.vector.bn_stats(stt[:, tb, g, :], xg[:, g, :])
                    nc.vector.bn_aggr(mv[:, tb, :], stt[:, tb, :, :])
                var = mv[:, :, 1:2].rearrange("p a b -> p (a b)")
                y = tmp[:, 0, :]
                if LN_ACT:
                    nc.scalar.activation(tmp[:, 1, :], var, AF.Ln, bias=eps_t[:], scale=1.0)
                    nc.scalar.activation(y, tmp[:, 1, :], AF.Exp, scale=-0.5)
                else:
                    # s = var + eps (f32), y0 = bits(0x5f3759df - (s_int >> 1))
                    nc.vector.tensor_scalar(
                        out=rs[:], in0=var, scalar1=LN_EPS, scalar2=None, op0=OP.add,
                    )
                    s_i = rs[:].bitcast(I32)
                    y_i = tmp[:, 0, :].bitcast(I32)
                    nc.vector.tensor_scalar(
                        out=y_i, in0=s_i, scalar1=1, scalar2=None,
                        op0=OP.logical_shift_right,
                    )
                    nc.vector.tensor_tensor(y_i, magic[:], y_i, OP.subtract)
                    t2 = tmp[:, 1, :]
                    for _ in range(2):
                        nc.vector.tensor_tensor(t2, y, y, OP.mult)
                        nc.vector.tensor_tensor(t2, t2, rs[:], OP.mult)
                        nc.vector.tensor_scalar(
                            out=t2, in0=t2, scalar1=-0.5, scalar2=1.5,
                            op0=OP.mult, op1=OP.add,
                        )
                        nc.vector.tensor_tensor(y, y, t2, OP.mult)
                for tb in range(NB_T):
                    nc.vector.tensor_scalar(
                        out=h_out[:, tb, :], in0=x_in[:, tb, :],
                        scalar1=mv[:, tb, 0:1], scalar2=y[:, tb:tb + 1],
                        op0=OP.subtract, op1=OP.mult,
                    )

            def transpose_tm_to_fm(h_fm, h_tm, h_dr, eng):
                """[128, 4, 768] bf16 token-major -> [128, 6, 512] bf16
                feature-major via a DRAM store + DMA xbar transposes."""
                for tb in range(NB_T):
                    eng.dma_start(h_dr[tb], h_tm[:, tb, :])
                ht_flat = h_dr.rearrange("k p n -> (k p) n")
                for db in range(NB_D):
                    eng.dma_start_transpose(
                        h_fm[:, db, :], ht_flat[:, db * P:(db + 1) * P]
                    )

            def transpose_tm_to_fm_pe(h_fm, h_tm, tagsfx):
                """Same transform on the PE (bf16), for when the PE would
                otherwise idle waiting on the DMA chain."""
                for db in range(NB_D):
                    trp = psB.tile([P, T_OWN], BF16, tag="psB", name=f"trp{tagsfx}_{db}")
                    for tb in range(NB_T):
                        nc.tensor.transpose(
                            trp[:, tb * P:(tb + 1) * P],
                            h_tm[:, tb, db * P:(db + 1) * P], idbf[:],
                        )
                    nc.vector.tensor_copy(h_fm[:, db, :], trp[:])

            # ---- initial stream ----
            x_t = sp.tile([P, NB_T, D], F32R, tag="stream", name="x0")
            nc.sync.dma_start(x_t[:], x_d[:])

            for l in range(L):
                # ---- weights for this layer ----
                wq = wp.tile([P, NB_D, D], BF16, tag="wbf", bufs=5, name=f"wq{l}")
                nc.sync.dma_start(wq[:], wq_d[l].rearrange("p (k n) -> p k n", n=D))
                wv = wp.tile([P, NB_D, D], BF16, tag="wbf", bufs=5, name=f"wv{l}")
                nc.sync.dma_start(wv[:], wv_d[l].rearrange("p (k n) -> p k n", n=D))
                bv_row = ap.tile([1, D], F32, tag="bvrow", bufs=2, name=f"bvr{l}")
                nc.sync.dma_start(bv_row[:], bvr_d[:, l, :])
                bo_row = ap.tile([1, D], BF16, tag="borow", bufs=2, name=f"bor{l}")
                nc.sync.dma_start(bo_row[:], bor_d[:, l, :])
                b2_row = ap.tile([1, D], BF16, tag="b2row", bufs=2, name=f"b2r{l}")
                nc.sync.dma_start(b2_row[:], b2r_d[:, l, :])

                # ---- LN1 + transpose ----
                with nc.named_scope(f"L{l:02d}_a_ln1"):
                    h_tm = ap.tile([P, NB_T, D], BF16, tag="h_tm", bufs=1, name=f"h1tm{l}")
                    ln_group(h_tm, x_t)
                    h1_fm = ap.tile([P, NB_D, T_OWN], BF16, tag="h_fm", bufs=1,
                                    name=f"h1fm{l}")
                    transpose_tm_to_fm_pe(h1_fm, h_tm, f"h1_{l}")

                # ---- V token-major (+bias) -> send buffer ----
                nc.enter_named_scope(f"L{l:02d}_b_v", False)
                bv_bc = ap.tile([P, D], F32, tag="bv_bc", bufs=1, name=f"bvbc{l}")
                nc.gpsimd.partition_broadcast(bv_bc[:], bv_row[:])
                v_send = ap.tile([P, NB_T, D], BF16, tag="vsend", bufs=1, name=f"vsend{l}")
                vsend_dr = dp.tile([NB_T, P, D], BF16, tag="vsend_d", name=f"vsdr{l}")
                vrecv_a = dp.tile([4, P, D], BF16, tag="vrecv_a", name=f"vra{l}")
                vrecv_b = dp.tile([4, P, D], BF16, tag="vrecv_b", name=f"vrb{l}")
                for tb in range(NB_T):
                    vp = psA.tile([P, D], F32, tag="psA", name=f"vps{l}_{tb}")
                    for n0, n1 in _regions():
                        for kb in range(NB_D):
                            nc.tensor.matmul(
                                vp[:, n0:n1],
                                h1_fm[:, kb, tb * P:(tb + 1) * P],
                                wv[:, kb, n0:n1],
                                start=(kb == 0), stop=(kb == NB_D - 1),
                            )
                    nc.vector.tensor_tensor(v_send[:, tb, :], vp[:], bv_bc[:], OP.add)
                    nc.sync.dma_start(vsend_dr[tb], v_send[:, tb, :])
                    if tb == 1:
                        nc.gpsimd.collective_compute(
                            "AllGather", OP.bypass, replica_groups=REPLICA_GROUPS,
                            ins=[vsend_dr[0:2]], outs=[vrecv_a[:]],
                        )
                    if tb == 3:
                        nc.gpsimd.collective_compute(
                            "AllGather", OP.bypass, replica_groups=REPLICA_GROUPS,
                            ins=[vsend_dr[2:4]], outs=[vrecv_b[:]],
                        )
                nc.leave_named_scope(f"L{l:02d}_b_v", None, False)
                nc.enter_named_scope(f"L{l:02d}_d_q", False)
                # ---- Q (overlaps the AllGather) ----
                for m in range(NB_D):
                    qp = psB.tile([P, T_OWN], F32, tag="psB", name=f"qps{l}_{m}")
                    for kb in range(NB_D):
                        nc.tensor.matmul(
                            qp[:], wq[:, kb, m * P:(m + 1) * P], h1_fm[:, kb, :],
                            start=(kb == 0), stop=(kb == NB_D - 1),
                        )
                    nc.vector.tensor_scalar(
                        out=q_par[:, m, :], in0=qp[:],
                        scalar1=bq_all[:, l, m:m + 1], scalar2=None,
                        op0=OP.add,
                    )

                # prefetch Wo while attention runs
                wo = wp.tile([P, NB_D, D], BF16, tag="wbf", bufs=5, name=f"wo{l}")
                nc.sync.dma_start(wo[:], wo_d[l].rearrange("p (k n) -> p k n", n=D))

                nc.leave_named_scope(f"L{l:02d}_d_q", None, False)
                nc.enter_named_scope(f"L{l:02d}_e_vrecv", False)
                # ---- receive V: augmented token-major + feature-major ----
                # own-relative key order: kb 0-3 = own half (local data),
                # kb 4-7 = peer half (dynamic shard pick from vrecv)
                v_fm = ap.tile([P, NB_D, NB_K * P], BF16, tag="v_fm", bufs=1, name=f"vfm{l}")
                for tb in range(NB_T):
                    nc.vector.tensor_copy(
                        va4[:, tb, :, 0:64],
                        v_send[:, tb, :].rearrange("p (h c) -> p h c", c=64),
                    )
                # own-half v_fm via PE transposes straight from SBUF: any DMA
                # here queues behind the AllGather's transfers and stalls attention
                for db in range(NB_D):
                    vto = psA.tile([P, T_OWN], BF16, tag="psA", name=f"vto{l}_{db}")
                    for tb in range(NB_T):
                        nc.tensor.transpose(
                            vto[:, tb * P:(tb + 1) * P],
                            v_send[:, tb, db * P:(db + 1) * P], idbf[:],
                        )
                    nc.vector.tensor_copy(v_fm[:, db, 0:T_OWN], vto[:])
                peer_off = (1 - (nc.gpsimd.partition_id() % 2)) * 2
                nc.gpsimd.dma_start(
                    va4[:, 4:6, :, 0:64],
                    vrecv_a.rearrange("k p (h c) -> k p h c", c=64)[ds(peer_off, 2)],
                )
                nc.gpsimd.dma_start(
                    va4[:, 6:8, :, 0:64],
                    vrecv_b.rearrange("k p (h c) -> k p h c", c=64)[ds(peer_off, 2)],
                )
                nc.leave_named_scope(f"L{l:02d}_e_vrecv", None, False)
                nc.enter_named_scope(f"L{l:02d}_f_attn", False)
                # ---- attention: flat software pipeline over (db, kb) steps;
                # ctx lags scores by one step; db 0-1 own-half steps overlap
                # the AllGather ----
                ctx_n = ap.tile([P, NB_D, T_OWN], BF16, tag="ctx_n", bufs=1, name=f"ctxn{l}")
                # step schedule: (db, kb, is_first_of_db, is_last_of_db)
                steps = []
                for db in (0, 1):
                    for kb in (0, 1, 2, 3):
                        steps.append((db, kb, kb == 0, False))
                steps.append("PASS_A")
                for db in (0, 1):
                    for kb in (4, 5):
                        steps.append((db, kb, False, False))
                steps.append("PASS_B")
                for db in (0, 1):
                    for kb in (6, 7):
                        steps.append((db, kb, False, kb == 7))
                for db in range(2, NB_D):
                    for kb in range(NB_K):
                        steps.append((db, kb, kb == 0, kb == NB_K - 1))

                ctxps = {}
                pending = None  # (db, kb, ex, is_first, is_last)

                def flush_ctx():
                    nonlocal pending
                    if pending is None:
                        return
                    db, kb, ex, first, last = pending
                    hpair = (2 * db, 2 * db + 1)
                    for hh in range(2):
                        nc.tensor.matmul(
                            ctxps[db][hh][:],
                            v_aug[:, kb, 65 * hpair[hh]:65 * hpair[hh] + 65],
                            ex[:, 512 * hh:512 * hh + 512],
                            start=first, stop=last,
                        )
                    if last:
                        for hh in range(2):
                            h = hpair[hh]
                            r0 = 64 * hh
                            zsc = ap.tile([1, T_OWN], F32, tag="zsc", bufs=2,
                                          name=f"zs{l}_{h}")
                            if SLOW_RECIP:
                                nc.vector.reciprocal(zsc[:], ctxps[db][hh][64:65, :])
                            else:
                                zrow = ap.tile([1, T_OWN], F32, tag="zrow", bufs=2,
                                               name=f"zr{l}_{h}")
                                nc.vector.tensor_copy(zrow[:], ctxps[db][hh][64:65, :])
                                nc.vector.reciprocal_approx_fast(
                                    out=zsc[:], in_=zrow[:],
                                )
                            zbc = ap.tile([P, T_OWN], F32, tag="zbc", bufs=2,
                                          name=f"zb{l}_{h}")
                            nc.gpsimd.partition_broadcast(zbc[:], zsc[:])
                            nc.vector.tensor_tensor(
                                ctx_n[r0:r0 + 64, db, :], ctxps[db][hh][0:64, :],
                                zbc[0:64, :], OP.mult,
                            )
                        del ctxps[db]
                    pending = None

                for stp in steps:
                    if stp in ("PASS_A", "PASS_B"):
                        # peer half of v_fm via PE transposes, per recv chunk
                        lo = 4 if stp == "PASS_A" else 6
                        c0 = (lo - 4) * P
                        for db in range(NB_D):
                            vtp = psA.tile([P, 2 * P], BF16, tag="psA",
                                           name=f"vt{stp}{l}_{db}")
                            for tb in (lo, lo + 1):
                                for hh in range(2):
                                    nc.tensor.transpose(
                                        vtp[64 * hh:64 * hh + 64,
                                            (tb - lo) * P:(tb - lo + 1) * P],
                                        v_aug[:, tb,
                                              65 * (2 * db + hh):65 * (2 * db + hh) + 64],
                                        idbf[:],
                                    )
                            nc.vector.tensor_copy(
                                v_fm[:, db, T_OWN + c0:T_OWN + c0 + 2 * P], vtp[:]
                            )
                        continue
                    db, kb, first, last = stp
                    if first:
                        ctxps[db] = [
                            psB.tile([65, T_OWN], F32, tag="psB",
                                     name=f"ctxp{l}_{2 * db + hh}")
                            for hh in range(2)
                        ]
                    spv = psA.tile([P, 2 * T_OWN], F32, tag="psA",
                                   name=f"sc{l}_{db}_{kb}")
                    for hh in range(2):
                        r0 = 64 * hh
                        nc.tensor.matmul(
                            spv[:, 512 * hh:512 * hh + 512],
                            v_fm[r0:r0 + 64, db, kb * P:(kb + 1) * P],
                            q_par[r0:r0 + 64, db, :],
                            start=True, stop=True,
                        )
                    flush_ctx()
                    ex = ap.tile([P, 2 * T_OWN], BF16, tag="expT", bufs=4,
                                 name=f"ex{l}_{db}_{kb}")
                    nc.scalar.activation(ex[:], spv[:], AF.Exp, scale=SCALE)
                    pending = (db, kb, ex, first, last)
                flush_ctx()
                nc.leave_named_scope(f"L{l:02d}_f_attn", None, False)
                nc.enter_named_scope(f"L{l:02d}_g_wo", False)
                # ---- Wo + residual (+ LN2 stats per block as it completes) ----
                skip = sp.tile([P, NB_T, D], F32R, tag="stream", name=f"skip{l}")
                h_tm2 = ap.tile([P, NB_T, D], BF16, tag="h_tm2", bufs=1, name=f"h2tm{l}")
                for lb in range(NB_T):
                    wps = psA.tile([P, D], F32, tag="psA", name=f"wops{l}_{lb}")
                    for n0, n1 in _regions():
                        for kb in range(NB_D):
                            nc.tensor.matmul(
                                wps[:, n0:n1],
                                ctx_n[:, kb, lb * P:(lb + 1) * P],
                                wo[:, kb, n0:n1],
                                start=(kb == 0), stop=False,
                            )
                        nc.tensor.matmul(
                            wps[:, n0:n1], ones1[:], bo_row[:, n0:n1],
                            start=False, stop=True,
                        )
                    nc.vector.tensor_tensor(skip[:, lb, :], x_t[:, lb, :], wps[:], OP.add)
                nc.leave_named_scope(f"L{l:02d}_g_wo", None, False)
                nc.enter_named_scope(f"L{l:02d}_h_ln2", False)
                # ---- LN2 + transpose ----
                ln_group(h_tm2, skip)
                h2_fm = ap.tile([P, NB_D, T_OWN], BF16, tag="h_fm2", bufs=1,
                                name=f"h2fm{l}")
                transpose_tm_to_fm_pe(h2_fm, h_tm2, f"h2_{l}")
                nc.leave_named_scope(f"L{l:02d}_h_ln2", None, False)
                nc.enter_named_scope(f"L{l:02d}_i_ff", False)
                # ---- FFN: FF1 (all 24 hidden blocks) then FF2 accumulated in PSUM ----
                g_all = ap.tile([P, NB_FF, T_OWN], BF16, tag="g", bufs=1, name=f"g{l}")
                w2cs = []
                for ck in range(4):
                    w1c = wp.tile([P, NB_D, D], BF16, tag="wbf", bufs=5, name=f"w1c{l}_{ck}")
                    nc.sync.dma_start(w1c[:], w1_d[l, ck].rearrange("p (k n) -> p k n", n=D))
                    w2c = wp.tile([P, NB_D, D], BF16, tag="wbf", bufs=5, name=f"w2c{l}_{ck}")
                    nc.sync.dma_start(w2c[:], w2_d[l, ck].rearrange("p (k n) -> p k n", n=D))
                    w2cs.append(w2c)
                    for mm in range(NB_D):
                        fp = psB.tile([P, T_OWN], F32, tag="psB", name=f"f1ps{l}_{ck}_{mm}")
                        for kb in range(NB_D):
                            nc.tensor.matmul(
                                fp[:], w1c[:, kb, mm * P:(mm + 1) * P], h2_fm[:, kb, :],
                                start=(kb == 0), stop=(kb == NB_D - 1),
                            )
                        nc.scalar.activation(
                            g_all[:, 6 * ck + mm, :], fp[:], AF.Gelu,
                            bias=b1_all[:, l, 6 * ck + mm:6 * ck + mm + 1], scale=1.0,
                        )
                for half in range(2):
                    f2s = []
                    for lb in (2 * half, 2 * half + 1):
                        f2 = psA.tile([P, D], F32, tag="psA", name=f"f2ps{l}_{lb}")
                        f2s.append(f2)
                        for n0, n1 in _regions():
                            for ck in range(4):
                                for mm in range(NB_D):
                                    nc.tensor.matmul(
                                        f2[:, n0:n1],
                                        g_all[:, 6 * ck + mm, lb * P:(lb + 1) * P],
                                        w2cs[ck][:, mm, n0:n1],
                                        start=(ck == 0 and mm == 0), stop=False,
                                    )
                            nc.tensor.matmul(
                                f2[:, n0:n1], ones1[:], b2_row[:, n0:n1],
                                start=False, stop=True,
                            )
                    for i, lb in enumerate((2 * half, 2 * half + 1)):
                        nc.vector.tensor_tensor(
                            skip[:, lb, :], skip[:, lb, :], f2s[i][:], OP.add,
                        )
                nc.leave_named_scope(f"L{l:02d}_i_ff", None, False)
                x_t = skip

            nc.sync.dma_start(out_d[:], x_t[:])
    nc.compile()
    return nc


def _preprocess(inputs, n_layers):
    """Fold LN affine into projections; lay out weights for tile DMA."""
    f32 = np.float32
    L = n_layers
    Wq = np.asarray(inputs["Wq"], f32)[:L]
    Wv = np.asarray(inputs["Wv"], f32)[:L]
    Wo = np.asarray(inputs["Wo"], f32)[:L]
    W1 = np.asarray(inputs["W1"], f32)[:L]
    W2 = np.asarray(inputs["W2"], f32)[:L]
    g1 = np.asarray(inputs["ln1_g"], f32)[:L]
    b1ln = np.asarray(inputs["ln1_b"], f32)[:L]
    g2 = np.asarray(inputs["ln2_g"], f32)[:L]
    b2ln = np.asarray(inputs["ln2_b"], f32)[:L]
    bq = np.asarray(inputs["bq"], f32)[:L]
    bv = np.asarray(inputs["bv"], f32)[:L]
    bo = np.asarray(inputs["bo"], f32)[:L]
    b1 = np.asarray(inputs["b1"], f32)[:L]
    b2 = np.asarray(inputs["b2"], f32)[:L]

    Wq_eff = g1[:, :, None] * Wq
    bq_eff = bq + np.einsum("ld,ldo->lo", b1ln, Wq)
    Wv_eff = g1[:, :, None] * Wv
    bv_eff = bv + np.einsum("ld,ldo->lo", b1ln, Wv)
    W1_eff = g2[:, :, None] * W1
    b1_eff = b1 + np.einsum("ld,ldo->lo", b2ln, W1)

    def fm_weight(W):  # [L, D, D] -> [L, 128, 6*768] with [p, k, n]
        return np.ascontiguousarray(
            W.reshape(L, NB_D, P, D).transpose(0, 2, 1, 3).reshape(L, P, NB_D * D)
        )

    bf = ml_dtypes.bfloat16
    wq_h = fm_weight(Wq_eff).astype(bf)
    wv_h = fm_weight(Wv_eff).astype(bf)
    wo_h = fm_weight(Wo).astype(bf)
    w1_h = np.ascontiguousarray(
        W1_eff.reshape(L, NB_D, P, 4, D).transpose(0, 3, 2, 1, 4).reshape(L, 4, P, NB_D * D)
    ).astype(bf)
    w2_h = np.ascontiguousarray(
        W2.reshape(L, 4, NB_D, P, D).transpose(0, 1, 3, 2, 4).reshape(L, 4, P, NB_D * D)
    ).astype(ml_dtypes.bfloat16)
    bq_h = np.ascontiguousarray(bq_eff.reshape(L, NB_D, P).transpose(2, 0, 1))
    b1_h = np.ascontiguousarray(b1_eff.reshape(L, NB_FF, P).transpose(2, 0, 1))

    return {
        "wq": wq_h, "wv": wv_h, "wo": wo_h, "w1": w1_h, "w2": w2_h,
        "bq": bq_h, "b1": b1_h,
        "bv_row": np.ascontiguousarray(bv_eff[None]),
        "bo_row": np.ascontiguousarray(bo[None]).astype(bf),
        "b2_row": np.ascontiguousarray(b2[None]).astype(bf),
        "identbf": np.eye(P).astype(ml_dtypes.bfloat16),
        "ones1": np.ones((1, P)).astype(bf),
    }


def kernel(**inputs) -> np.ndarray:
    n_layers = N_LAYERS
    key = ("nc", n_layers)
    if key not in _cached:
        _cached[key] = build(n_layers)
    nc = _cached[key]

    shared = _preprocess(inputs, n_layers)
    x = np.asarray(inputs["x"], np.float32)  # [4, 1024, 768]
    B, T, _ = x.shape

    in_maps = []
    for c in range(8):
        b, half = c // 2, c % 2
        x_own = x[b, half * T_OWN:(half + 1) * T_OWN]          # [512, 768]
        x_tile = np.ascontiguousarray(
            x_own.reshape(NB_T, P, D).transpose(1, 0, 2)        # [128, 4, 768]
        )
        in_maps.append({**shared, "x": x_tile})

    trace = bool(int(os.environ.get("KERNEL_TRACE", "0")))
    if trace:
        _register_ntff_hook()
    res = run_bass_kernel_spmd(nc, in_maps, core_ids=list(range(8)), trace=trace)
    global _last_results
    _last_results = res

    out = np.empty((B, T, D), dtype=np.float32)
    for c in range(8):
        b, half = c // 2, c % 2
        o = res.results[c]["out"]                               # [128, 4, 768]
        out[b, half * T_OWN:(half + 1) * T_OWN] = (
            o.transpose(1, 0, 2).reshape(T_OWN, D)
        )
    return out
